# revision 25
# baseline (speedup 1.0000x reference)
"""Causal multi-head self-attention on 8 Trainium2 NeuronCores.

Problem (hardcoded): x [2, 2048, 1024] f32, Wq/Wk/Wv/Wo [1024, 1024] f32,
H=16 heads, Dh=64, causal softmax(QK^T/8)V then output projection.

Sharding (Megatron-style): 2-way data parallel over batch x 4-way tensor
parallel over heads.  Core c handles batch c//4 and heads 4*(c%4)..+3 (a
256-wide slice of the hidden dim).  Wq/Wk/Wv sliced column-wise, Wo
row-wise; each core emits a partial [2048, 1024] output (bf16) which the
host sums per batch.

Device dataflow per core (single fused pipeline, ascending q-blocks):
  - x^T supplied host-side (d on partitions); per-(dc, qn) column-sliced
    DMAs so block 0's inputs land first
  - per q-block qn: Q/K/V projections for the *next* block are batched
    between this block's attention and epilogue, so TensorE never waits
    on DMA and ScalarE exp overlaps projection matmuls
  - scores computed transposed S^T[k, q], 2 heads row-packed in the PE
    array (Dh=64 contraction); straddle (diagonal) key-tiles compute only
    the valid column range [lo:512)
  - triangular causal mask added into PSUM via identity-matmul of a
    single [128,128] bf16 tile, only on the 128-wide diagonal sub-block
  - one exp() per PSUM tile on ScalarE (scale=1/8 folded in; straddles
    use a single strided sliced activation)
  - A.V with stationary [V | ones] so the softmax denominator appears as
    row 64 of the same matmul output; straddle key-tiles write only the
    valid column range (start flag on kt=0 which is always full-width)
  - normalize: K=1 ones-matmul broadcast + DVE reciprocal + tensor_mul
  - Wo row-parallel per 128-row output tile; Wo matmuls of block qn are
    spread into block qn+1's key-tile loop to fill exp-wait bubbles
"""

import os
import sys
from contextlib import ExitStack

import numpy as np

try:
    import concourse.bass as bass
except ImportError:  # pragma: no cover - path fallback for fresh dirs
    for p in ("/opt/trn_rl_repo", "/root/.axon_site/_ro/trn_rl_repo"):
        if os.path.isdir(p) and p not in sys.path:
            sys.path.insert(0, p)
    import concourse.bass as bass

import ml_dtypes
import concourse.bacc as bacc
import concourse.mybir as mybir
import concourse.tile as tile
from concourse.bass_utils import run_bass_kernel_spmd

F32 = mybir.dt.float32
F32R = mybir.dt.float32r
BF16 = mybir.dt.bfloat16

KCFG = os.environ.get("KCFG", "faster")
_DT = {
    "fast": dict(proj=BF16, qk=BF16, av=BF16, wo=F32R),
    "faster": dict(proj=BF16, qk=BF16, av=BF16, wo=BF16),
}[KCFG]
KRECIP = os.environ.get("KRECIP", "fast")

B, S, D = 2, 2048, 1024
H, DH = 16, 64
NCORES = 8
HPC = 4          # heads per core
JPC = HPC * DH   # 256 hidden dims per core
QB = 512         # query block
KB = 128         # key tile
NQ = S // QB     # 4
NK = S // KB     # 16
MASK_VAL = -1e7

_CACHE = {}
LAST_RESULTS = None


def _np_dt(dt):
    return ml_dtypes.bfloat16 if dt == BF16 else np.float32


def _build_nc():
    proj_dt, qk_dt, av_dt, wo_dt = _DT["proj"], _DT["qk"], _DT["av"], _DT["wo"]
    nc = bacc.Bacc()
    xT = nc.dram_tensor("xT", [D, S], proj_dt, kind="ExternalInput")
    wqT = nc.dram_tensor("wqT", [D, JPC], proj_dt, kind="ExternalInput")
    wkT = nc.dram_tensor("wkT", [D, JPC], proj_dt, kind="ExternalInput")
    wvT = nc.dram_tensor("wvT", [D, JPC], proj_dt, kind="ExternalInput")
    woT = nc.dram_tensor("woT", [JPC, D], wo_dt, kind="ExternalInput")
    tri = nc.dram_tensor("tri", [KB, KB], BF16, kind="ExternalInput")
    sel2 = nc.dram_tensor("sel2", [2, KB], F32R, kind="ExternalInput")
    ident = nc.dram_tensor("ident", [KB, KB], BF16, kind="ExternalInput")
    y = nc.dram_tensor("y", [S, D], BF16, kind="ExternalOutput")

    with tile.TileContext(nc) as tc:
        with (
            tc.tile_pool(name="const", bufs=1) as constp,
            tc.tile_pool(name="act", bufs=1) as actp,
            tc.tile_pool(name="e", bufs=8) as ep,
            tc.tile_pool(name="ps", bufs=2, space="PSUM") as psp,
            tc.tile_pool(name="avp", bufs=4, space="PSUM") as avp,
        ):
            ident_sb = constp.tile([KB, KB], BF16)
            tri_sb = constp.tile([KB, KB], BF16)
            wo_sb = actp.tile([128, 2, D], wo_dt)
            xT_sb = actp.tile([128, 8, S], proj_dt)
            wq_sb = actp.tile([128, 8, JPC], proj_dt)
            wk_sb = actp.tile([128, 8, JPC], proj_dt)
            wv_sb = actp.tile([128, 8, JPC], proj_dt)
            # QT/KT: [128, S] pair tiles; rows 0:64 head 2*pi, 64:128 2*pi+1
            QT = [actp.tile([128, S], qk_dt, name=f"QT{i}") for i in range(2)]
            KT = [actp.tile([128, S], qk_dt, name=f"KT{i}") for i in range(2)]
            # V with ones column appended per (k-tile, head): the softmax
            # denominator falls out of the A.V matmul as row 64
            V1 = actp.tile([128, NK, HPC, DH + 1], av_dt)
            OT = [actp.tile([128, S], av_dt, name=f"OT{i}") for i in range(2)]
            sums_sb = actp.tile([1, HPC, S], F32R, name="sums_sb")
            rsums_sb = actp.tile([1, HPC, S], F32R, name="rsums_sb")
            sel0_sb = constp.tile([1, KB], F32R)
            sel1_sb = constp.tile([1, KB], F32R)

            # ---------------- DMA issue ----------------
            # Per-queue DMA bandwidth is ~110GB/s: spread the 6MB of input
            # across sync/scalar/gpsimd/vector so block 0's operands (wq,
            # wk, wv, xT qn0, tri) land in parallel.
            wqR = wqT.rearrange("(c p) j -> p c j", p=128)
            wkR = wkT.rearrange("(c p) j -> p c j", p=128)
            nc.sync.dma_start(out=wq_sb[:, :, 0:128], in_=wqR[:, :, 0:128])
            nc.scalar.dma_start(out=wq_sb[:, :, 128:256], in_=wqR[:, :, 128:256])
            nc.gpsimd.dma_start(
                out=wv_sb[:], in_=wvT.rearrange("(c p) j -> p c j", p=128)
            )
            nc.gpsimd.dma_start(out=tri_sb[:], in_=tri[:])
            nc.gpsimd.dma_start(out=ident_sb[:], in_=ident[:])
            nc.gpsimd.dma_start(out=sel0_sb[:], in_=sel2[0:1, :])
            nc.gpsimd.dma_start(out=sel1_sb[:], in_=sel2[1:2, :])
            nc.gpsimd.memset(V1[:, :, :, DH : DH + 1], 1.0)
            for dc in range(8):
                eng = nc.sync if dc % 2 == 0 else nc.scalar
                eng.dma_start(
                    out=xT_sb[:, dc, 0:QB],
                    in_=xT[dc * 128 : (dc + 1) * 128, 0:QB],
                )
            nc.sync.dma_start(out=wk_sb[:, :, 0:128], in_=wkR[:, :, 0:128])
            nc.scalar.dma_start(out=wk_sb[:, :, 128:256], in_=wkR[:, :, 128:256])
            for dc in range(8):
                eng = nc.sync if dc % 2 == 0 else nc.scalar
                eng.dma_start(
                    out=xT_sb[:, dc, QB : 2 * QB],
                    in_=xT[dc * 128 : (dc + 1) * 128, QB : 2 * QB],
                )
            nc.gpsimd.dma_start(
                out=wo_sb[:], in_=woT.rearrange("(c p) j -> p c j", p=128)
            )
            for dc in range(8):
                nc.gpsimd.dma_start(
                    out=xT_sb[:, dc, 2 * QB : 3 * QB],
                    in_=xT[dc * 128 : (dc + 1) * 128, 2 * QB : 3 * QB],
                )
            for dc in range(8):
                nc.sync.dma_start(
                    out=xT_sb[:, dc, 3 * QB : 4 * QB],
                    in_=xT[dc * 128 : (dc + 1) * 128, 3 * QB : 4 * QB],
                )

            # ---------------- helpers ----------------
            def qk_group(w_sb, out_tiles, mj, qn):
                ps = psp.tile([128, 1024], F32, tag="mm", name="ps_qk")
                for dc in range(8):
                    nc.tensor.matmul(
                        ps[:, :QB],
                        lhsT=w_sb[:, dc, mj * 128 : (mj + 1) * 128],
                        rhs=xT_sb[:, dc, qn * QB : (qn + 1) * QB],
                        start=(dc == 0),
                        stop=(dc == 7),
                    )
                nc.vector.tensor_copy(
                    out_tiles[mj][:, qn * QB : (qn + 1) * QB], ps[:, :QB]
                )

            def v_group(st):
                ps = psp.tile([128, 1024], F32, tag="mm", name="ps_v")
                for dc in range(8):
                    nc.tensor.matmul(
                        ps[:, :JPC],
                        lhsT=xT_sb[:, dc, st * 128 : (st + 1) * 128],
                        rhs=wv_sb[:, dc, :],
                        start=(dc == 0),
                        stop=(dc == 7),
                    )
                nc.vector.tensor_copy(
                    V1[:, st, :, 0:DH],
                    ps[:, :JPC].rearrange("p (h d) -> p h d", h=HPC),
                )

            def proj_block(qn):
                for mj in range(2):
                    qk_group(wq_sb, QT, mj, qn)
                for mj in range(2):
                    qk_group(wk_sb, KT, mj, qn)
                for st in range(4 * qn, 4 * qn + 4):
                    v_group(st)

            def emit_scores_exp(qn, kt):
                """Returns E tile pair for this key tile."""
                straddle = kt >= 4 * qn
                lo = 128 * (kt - 4 * qn) if straddle else 0
                E = []
                for pi in range(2):
                    ps = psp.tile([128, 1024], F32, tag="mm", name="ps_sc")
                    for hh in range(2):
                        nc.tensor.matmul(
                            ps[:, hh * QB + lo : (hh + 1) * QB],
                            lhsT=KT[pi][
                                hh * 64 : (hh + 1) * 64,
                                kt * KB : (kt + 1) * KB,
                            ],
                            rhs=QT[pi][
                                hh * 64 : (hh + 1) * 64,
                                qn * QB + lo : (qn + 1) * QB,
                            ],
                            start=True,
                            stop=not straddle,
                            tile_position=(hh * 64, 0),
                        )
                    if straddle:
                        for hh in range(2):
                            nc.tensor.matmul(
                                ps[:, hh * QB + lo : hh * QB + lo + 128],
                                lhsT=ident_sb,
                                rhs=tri_sb,
                                start=False,
                                stop=True,
                            )
                    e = ep.tile([128, 1024], av_dt, tag="e", name="e")
                    if straddle and lo > 0:
                        # contiguous per-hh activations: strided 2-row APs
                        # cost ~2x on ScalarE
                        for hh in range(2):
                            nc.scalar.activation(
                                e[:, hh * QB + lo : (hh + 1) * QB],
                                ps[:, hh * QB + lo : (hh + 1) * QB],
                                mybir.ActivationFunctionType.Exp,
                                scale=0.125,
                            )
                    else:
                        nc.scalar.activation(
                            e[:],
                            ps[:],
                            mybir.ActivationFunctionType.Exp,
                            scale=0.125,
                        )
                    E.append(e)
                return E

            def emit_av(qn, kt, E, av, nkt):
                straddle = kt >= 4 * qn
                lo = 128 * (kt - 4 * qn) if straddle else 0
                first, last = kt == 0, kt == nkt - 1
                for h in range(HPC):
                    p, j = h // 2, h % 2
                    nc.tensor.matmul(
                        av[h][:, lo:QB],
                        lhsT=V1[:, kt, h, :],
                        rhs=E[p][:, j * QB + lo : (j + 1) * QB],
                        start=first,
                        stop=last,
                    )

            def emit_wo(qn, st, last_block):
                ps_y = psp.tile([128, 1024], F32, tag="mm", name="ps_y")
                for nn in range(2):
                    for p in range(2):
                        nc.tensor.matmul(
                            ps_y[:, nn * QB : (nn + 1) * QB],
                            lhsT=OT[p][:, st * 128 : (st + 1) * 128],
                            rhs=wo_sb[:, p, nn * QB : (nn + 1) * QB],
                            start=(p == 0),
                            stop=(p == 1),
                        )
                y_sb = latep.tile([128, D], av_dt, tag="y", bufs=3, name="y_sb")
                if last_block:
                    # ScalarE is free of exps by now; spread the final DMAs
                    # over all three queues so the drain isn't queue-bound
                    nc.scalar.copy(y_sb[:], ps_y[:])
                    oeng = (nc.sync, nc.scalar, nc.gpsimd)[st % 3]
                else:
                    nc.vector.tensor_copy(y_sb[:], ps_y[:])
                    oeng = nc.sync if st % 2 == 0 else nc.scalar
                oeng.dma_start(out=y[st * 128 : (st + 1) * 128, :], in_=y_sb[:])

            def epilogue_norm(qn, av):
                qs = slice(qn * QB, (qn + 1) * QB)
                for h in range(HPC):
                    p, j = h // 2, h % 2
                    nc.vector.tensor_copy(
                        OT[p][j * 64 : (j + 1) * 64, qs], av[h][0:DH, :]
                    )
                    nc.vector.tensor_copy(
                        sums_sb[0:1, h, qs], av[h][DH : DH + 1, :]
                    )
                # reciprocal on the [1, 512] sums rows first (placing head
                # 2p+j's reciprocals on partition j), then one K=2 matmul
                # per pi with a block-selector stationary broadcasts them
                # to [0:64] / [64:128]
                with nc.allow_low_precision(reason="f32r is f32-width"):
                    for h in range(HPC):
                        nc.vector.reciprocal(
                            rsums_sb[0:1, h, qs], sums_sb[0:1, h, qs]
                        )
                for p in range(2):
                    rb_ps = avp.tile([128, QB], F32, tag="av", name="rb_ps")
                    for j, sel in ((0, sel0_sb), (1, sel1_sb)):
                        nc.tensor.matmul(
                            rb_ps[:],
                            lhsT=sel[:],
                            rhs=rsums_sb[0:1, 2 * p + j, qs],
                            start=(j == 0),
                            stop=(j == 1),
                        )
                    nc.vector.tensor_mul(OT[p][:, qs], OT[p][:, qs], rb_ps[:])

            # ---------------- pipelined main loop ----------------
            late_ctx = ExitStack()
            latep = late_ctx.enter_context(tc.tile_pool(name="late", bufs=1))

            proj_block(0)
            pending_wo = []  # (qn, st) of deferred output projections
            for qn in range(NQ):
                nkt = 4 * qn + 4
                last_block = qn == NQ - 1
                av = [
                    avp.tile([DH + 1, QB], F32, tag="av", name=f"av{h}")
                    for h in range(HPC)
                ]
                prevE = None
                for kt in range(nkt):
                    if pending_wo and kt in (2, 5, 8, 11):
                        emit_wo(*pending_wo.pop(0), last_block=False)
                    E = emit_scores_exp(qn, kt)
                    if prevE is not None:
                        emit_av(qn, kt - 1, prevE, av, nkt)
                    prevE = E
                emit_av(qn, nkt - 1, prevE, av, nkt)
                # leftover deferred Wo (short blocks have few kt slots)
                while pending_wo:
                    emit_wo(*pending_wo.pop(0), last_block=False)
                if not last_block:
                    proj_block(qn + 1)
                epilogue_norm(qn, av)
                for st in range(4 * qn, 4 * qn + 4):
                    if last_block:
                        emit_wo(qn, st, last_block=True)
                    else:
                        pending_wo.append((qn, st))
            while pending_wo:
                emit_wo(*pending_wo.pop(0), last_block=True)
            late_ctx.close()
    return nc


def _get_nc():
    if "nc" not in _CACHE:
        nc = _build_nc()
        nc.finalize()
        _CACHE["nc"] = nc
    return _CACHE["nc"]


def _host_consts():
    rk = np.arange(KB)[:, None]
    rq = np.arange(KB)[None, :]
    tri = np.where(rq >= rk, 0.0, MASK_VAL).astype(ml_dtypes.bfloat16)
    identity = np.eye(KB, dtype=ml_dtypes.bfloat16)
    return tri, identity


def kernel(x, Wq, Wk, Wv, Wo):
    global LAST_RESULTS
    x = np.asarray(x, np.float32)
    Wq = np.asarray(Wq, np.float32)
    Wk = np.asarray(Wk, np.float32)
    Wv = np.asarray(Wv, np.float32)
    Wo = np.asarray(Wo, np.float32)

    pdt, wdt = _np_dt(_DT["proj"]), _np_dt(_DT["wo"])
    tri, identity = _host_consts()
    sel2_np = np.zeros((2, KB), np.float32)
    sel2_np[0, 0:64] = 1.0
    sel2_np[1, 64:128] = 1.0
    xTs = [np.ascontiguousarray(x[b].T).astype(pdt) for b in range(B)]

    in_maps = []
    for c in range(NCORES):
        b, g = c // (NCORES // B), c % (NCORES // B)
        jsel = slice(g * JPC, (g + 1) * JPC)
        in_maps.append(
            {
                "xT": xTs[b],
                "wqT": np.ascontiguousarray(Wq[jsel].T).astype(pdt),
                "wkT": np.ascontiguousarray(Wk[jsel].T).astype(pdt),
                "wvT": np.ascontiguousarray(Wv[jsel].T).astype(pdt),
                "woT": np.ascontiguousarray(Wo[:, jsel].T).astype(wdt),
                "tri": tri,
                "sel2": sel2_np,
                "ident": identity,
            }
        )

    res = run_bass_kernel_spmd(_get_nc(), in_maps, list(range(NCORES)))
    LAST_RESULTS = res
    ys = [res.results[c]["y"].astype(np.float32) for c in range(NCORES)]
    npc = NCORES // B
    out = np.stack(
        [sum(ys[b * npc + 1 : (b + 1) * npc], ys[b * npc]) for b in range(B)]
    )
    return out.astype(np.float32)


# revision 28
# speedup vs baseline: 1.4815x; 1.4815x over previous
"""Causal multi-head self-attention on 8 Trainium2 NeuronCores.

Problem (hardcoded): x [2, 2048, 1024] f32, Wq/Wk/Wv/Wo [1024, 1024] f32,
H=16 heads, Dh=64, causal softmax(QK^T/8)V then output projection.

Sharding (Megatron-style): 2-way data parallel over batch x 4-way tensor
parallel over heads.  Core c handles batch c//4 and heads 4*(c%4)..+3 (a
256-wide slice of the hidden dim).  Wq/Wk/Wv sliced column-wise, Wo
row-wise; each core emits a partial [2048, 1024] output (bf16) which the
host sums per batch.

Device dataflow per core (single fused pipeline, ascending q-blocks):
  - x^T supplied host-side (d on partitions); per-(dc, qn) column-sliced
    DMAs so block 0's inputs land first
  - per q-block qn: Q/K/V projections for the *next* block are batched
    between this block's attention and epilogue, so TensorE never waits
    on DMA and ScalarE exp overlaps projection matmuls
  - scores computed transposed S^T[k, q], 2 heads row-packed in the PE
    array (Dh=64 contraction); straddle (diagonal) key-tiles compute only
    the valid column range [lo:512)
  - triangular causal mask added into PSUM via identity-matmul of a
    single [128,128] bf16 tile, only on the 128-wide diagonal sub-block
  - one exp() per PSUM tile on ScalarE (scale=1/8 folded in; straddles
    use a single strided sliced activation)
  - A.V with stationary [V | ones] so the softmax denominator appears as
    row 64 of the same matmul output; straddle key-tiles write only the
    valid column range (start flag on kt=0 which is always full-width)
  - normalize: K=1 ones-matmul broadcast + DVE reciprocal + tensor_mul
  - Wo row-parallel per 128-row output tile; Wo matmuls of block qn are
    spread into block qn+1's key-tile loop to fill exp-wait bubbles
"""

import os
import sys
from contextlib import ExitStack

import numpy as np

try:
    import concourse.bass as bass
except ImportError:  # pragma: no cover - path fallback for fresh dirs
    for p in ("/opt/trn_rl_repo", "/root/.axon_site/_ro/trn_rl_repo"):
        if os.path.isdir(p) and p not in sys.path:
            sys.path.insert(0, p)
    import concourse.bass as bass

import ml_dtypes
import concourse.bacc as bacc
import concourse.mybir as mybir
import concourse.tile as tile
from concourse.bass_utils import run_bass_kernel_spmd

F32 = mybir.dt.float32
F32R = mybir.dt.float32r
BF16 = mybir.dt.bfloat16

KCFG = os.environ.get("KCFG", "faster")
_DT = {
    "fast": dict(proj=BF16, qk=BF16, av=BF16, wo=F32R),
    "faster": dict(proj=BF16, qk=BF16, av=BF16, wo=BF16),
}[KCFG]
KRECIP = os.environ.get("KRECIP", "fast")

B, S, D = 2, 2048, 1024
H, DH = 16, 64
NCORES = 8
HPC = 4          # heads per core
JPC = HPC * DH   # 256 hidden dims per core
QB = 512         # query block
KB = 128         # key tile
NQ = S // QB     # 4
NK = S // KB     # 16
MASK_VAL = -1e7

_CACHE = {}
LAST_RESULTS = None


def _np_dt(dt):
    return ml_dtypes.bfloat16 if dt == BF16 else np.float32


def _build_nc():
    proj_dt, qk_dt, av_dt, wo_dt = _DT["proj"], _DT["qk"], _DT["av"], _DT["wo"]
    nc = bacc.Bacc()
    xT = nc.dram_tensor("xT", [D, S], proj_dt, kind="ExternalInput")
    wqT = nc.dram_tensor("wqT", [D, JPC], proj_dt, kind="ExternalInput")
    wkT = nc.dram_tensor("wkT", [D, JPC], proj_dt, kind="ExternalInput")
    wvT = nc.dram_tensor("wvT", [D, JPC], proj_dt, kind="ExternalInput")
    woT = nc.dram_tensor("woT", [JPC, D], wo_dt, kind="ExternalInput")
    tri = nc.dram_tensor("tri", [KB, KB], BF16, kind="ExternalInput")
    sel2 = nc.dram_tensor("sel2", [2, KB], F32R, kind="ExternalInput")
    ident = nc.dram_tensor("ident", [KB, KB], BF16, kind="ExternalInput")
    y = nc.dram_tensor("y", [S, D], BF16, kind="ExternalOutput")

    with tile.TileContext(nc) as tc:
        with (
            tc.tile_pool(name="const", bufs=1) as constp,
            tc.tile_pool(name="act", bufs=1) as actp,
            tc.tile_pool(name="e", bufs=8) as ep,
            tc.tile_pool(name="ps", bufs=2, space="PSUM") as psp,
            tc.tile_pool(name="avp", bufs=4, space="PSUM") as avp,
        ):
            ident_sb = constp.tile([KB, KB], BF16)
            tri_sb = constp.tile([KB, KB], BF16)
            wo_sb = actp.tile([128, 2, D], wo_dt)
            xT_sb = actp.tile([128, 8, S], proj_dt)
            wq_sb = actp.tile([128, 8, JPC], proj_dt)
            wk_sb = actp.tile([128, 8, JPC], proj_dt)
            wv_sb = actp.tile([128, 8, JPC], proj_dt)
            # QT/KT: [128, S] pair tiles; rows 0:64 head 2*pi, 64:128 2*pi+1
            QT = [actp.tile([128, S], qk_dt, name=f"QT{i}") for i in range(2)]
            KT = [actp.tile([128, S], qk_dt, name=f"KT{i}") for i in range(2)]
            # V with ones column appended per (k-tile, head): the softmax
            # denominator falls out of the A.V matmul as row 64
            V1 = actp.tile([128, NK, HPC, DH + 1], av_dt)
            OT = [actp.tile([128, S], av_dt, name=f"OT{i}") for i in range(2)]
            sums_sb = actp.tile([1, HPC, S], F32R, name="sums_sb")
            sel0_sb = constp.tile([1, KB], F32R)
            sel1_sb = constp.tile([1, KB], F32R)

            # ---------------- DMA issue ----------------
            # Per-queue DMA bandwidth is ~110GB/s: spread the 6MB of input
            # across sync/scalar/gpsimd/vector so block 0's operands (wq,
            # wk, wv, xT qn0, tri) land in parallel.
            wqR = wqT.rearrange("(c p) j -> p c j", p=128)
            wkR = wkT.rearrange("(c p) j -> p c j", p=128)
            nc.sync.dma_start(out=wq_sb[:, :, 0:128], in_=wqR[:, :, 0:128])
            nc.scalar.dma_start(out=wq_sb[:, :, 128:256], in_=wqR[:, :, 128:256])
            nc.gpsimd.dma_start(
                out=wv_sb[:], in_=wvT.rearrange("(c p) j -> p c j", p=128)
            )
            nc.gpsimd.dma_start(out=tri_sb[:], in_=tri[:])
            nc.gpsimd.dma_start(out=ident_sb[:], in_=ident[:])
            nc.gpsimd.dma_start(out=sel0_sb[:], in_=sel2[0:1, :])
            nc.gpsimd.dma_start(out=sel1_sb[:], in_=sel2[1:2, :])
            nc.gpsimd.memset(V1[:, :, :, DH : DH + 1], 1.0)
            for dc in range(8):
                eng = nc.sync if dc % 2 == 0 else nc.scalar
                eng.dma_start(
                    out=xT_sb[:, dc, 0:QB],
                    in_=xT[dc * 128 : (dc + 1) * 128, 0:QB],
                )
            nc.sync.dma_start(out=wk_sb[:, :, 0:128], in_=wkR[:, :, 0:128])
            nc.scalar.dma_start(out=wk_sb[:, :, 128:256], in_=wkR[:, :, 128:256])
            for dc in range(8):
                eng = nc.sync if dc % 2 == 0 else nc.scalar
                eng.dma_start(
                    out=xT_sb[:, dc, QB : 2 * QB],
                    in_=xT[dc * 128 : (dc + 1) * 128, QB : 2 * QB],
                )
            nc.gpsimd.dma_start(
                out=wo_sb[:], in_=woT.rearrange("(c p) j -> p c j", p=128)
            )
            for dc in range(8):
                nc.gpsimd.dma_start(
                    out=xT_sb[:, dc, 2 * QB : 3 * QB],
                    in_=xT[dc * 128 : (dc + 1) * 128, 2 * QB : 3 * QB],
                )
            for dc in range(8):
                nc.sync.dma_start(
                    out=xT_sb[:, dc, 3 * QB : 4 * QB],
                    in_=xT[dc * 128 : (dc + 1) * 128, 3 * QB : 4 * QB],
                )

            # ---------------- helpers ----------------
            def qk_group(w_sb, out_tiles, mj, qn):
                ps = psp.tile([128, 1024], F32, tag="mm", name="ps_qk")
                for dc in range(8):
                    nc.tensor.matmul(
                        ps[:, :QB],
                        lhsT=w_sb[:, dc, mj * 128 : (mj + 1) * 128],
                        rhs=xT_sb[:, dc, qn * QB : (qn + 1) * QB],
                        start=(dc == 0),
                        stop=(dc == 7),
                    )
                nc.vector.tensor_copy(
                    out_tiles[mj][:, qn * QB : (qn + 1) * QB], ps[:, :QB]
                )

            def v_group(st):
                ps = psp.tile([128, 1024], F32, tag="mm", name="ps_v")
                for dc in range(8):
                    nc.tensor.matmul(
                        ps[:, :JPC],
                        lhsT=xT_sb[:, dc, st * 128 : (st + 1) * 128],
                        rhs=wv_sb[:, dc, :],
                        start=(dc == 0),
                        stop=(dc == 7),
                    )
                nc.vector.tensor_copy(
                    V1[:, st, :, 0:DH],
                    ps[:, :JPC].rearrange("p (h d) -> p h d", h=HPC),
                )

            def proj_block(qn):
                for mj in range(2):
                    qk_group(wq_sb, QT, mj, qn)
                for mj in range(2):
                    qk_group(wk_sb, KT, mj, qn)
                for st in range(4 * qn, 4 * qn + 4):
                    v_group(st)

            def emit_scores_exp(qn, kt):
                """Returns E tile pair for this key tile."""
                straddle = kt >= 4 * qn
                lo = 128 * (kt - 4 * qn) if straddle else 0
                E = []
                for pi in range(2):
                    ps = psp.tile([128, 1024], F32, tag="mm", name="ps_sc")
                    for hh in range(2):
                        nc.tensor.matmul(
                            ps[:, hh * QB + lo : (hh + 1) * QB],
                            lhsT=KT[pi][
                                hh * 64 : (hh + 1) * 64,
                                kt * KB : (kt + 1) * KB,
                            ],
                            rhs=QT[pi][
                                hh * 64 : (hh + 1) * 64,
                                qn * QB + lo : (qn + 1) * QB,
                            ],
                            start=True,
                            stop=not straddle,
                            tile_position=(hh * 64, 0),
                        )
                    if straddle:
                        for hh in range(2):
                            nc.tensor.matmul(
                                ps[:, hh * QB + lo : hh * QB + lo + 128],
                                lhsT=ident_sb,
                                rhs=tri_sb,
                                start=False,
                                stop=True,
                            )
                    e = ep.tile([128, 1024], av_dt, tag="e", name="e")
                    # full-tile exp even for straddles: ScalarE has a large
                    # fixed cost per ACTIVATE, so one big op beats slices.
                    # Columns below the causal cutoff hold exp(stale PSUM);
                    # the sliced A.V matmuls never read them.
                    nc.scalar.activation(
                        e[:],
                        ps[:],
                        mybir.ActivationFunctionType.Exp,
                        scale=0.125,
                    )
                    E.append(e)
                return E

            def emit_av(qn, kt, E, av, nkt):
                straddle = kt >= 4 * qn
                lo = 128 * (kt - 4 * qn) if straddle else 0
                first, last = kt == 0, kt == nkt - 1
                for h in range(HPC):
                    p, j = h // 2, h % 2
                    nc.tensor.matmul(
                        av[h][:, lo:QB],
                        lhsT=V1[:, kt, h, :],
                        rhs=E[p][:, j * QB + lo : (j + 1) * QB],
                        start=first,
                        stop=last,
                    )

            def emit_wo(qn, st, last_block):
                ps_y = psp.tile([128, 1024], F32, tag="mm", name="ps_y")
                for nn in range(2):
                    for p in range(2):
                        nc.tensor.matmul(
                            ps_y[:, nn * QB : (nn + 1) * QB],
                            lhsT=OT[p][:, st * 128 : (st + 1) * 128],
                            rhs=wo_sb[:, p, nn * QB : (nn + 1) * QB],
                            start=(p == 0),
                            stop=(p == 1),
                        )
                y_sb = latep.tile([128, D], av_dt, tag="y", bufs=3, name="y_sb")
                if last_block:
                    # ScalarE is free of exps by now; spread the final DMAs
                    # over all three queues so the drain isn't queue-bound
                    nc.scalar.copy(y_sb[:], ps_y[:])
                    oeng = (nc.sync, nc.scalar, nc.gpsimd)[st % 3]
                else:
                    nc.vector.tensor_copy(y_sb[:], ps_y[:])
                    oeng = nc.sync if st % 2 == 0 else nc.scalar
                oeng.dma_start(out=y[st * 128 : (st + 1) * 128, :], in_=y_sb[:])

            def epilogue_norm(qn, av):
                qs = slice(qn * QB, (qn + 1) * QB)
                for h in range(HPC):
                    p, j = h // 2, h % 2
                    nc.vector.tensor_copy(
                        OT[p][j * 64 : (j + 1) * 64, qs], av[h][0:DH, :]
                    )
                    nc.vector.tensor_copy(
                        sums_sb[0:1, h, qs], av[h][DH : DH + 1, :]
                    )
                # broadcast the raw sums to [0:64]/[64:128] with two K=1
                # selector matmuls, then one 128-lane approx reciprocal
                # (single-partition DVE reciprocals run on 1 lane: ~4us!)
                for p in range(2):
                    rb_ps = avp.tile([128, QB], F32, tag="av", name="rb_ps")
                    for j, sel in ((0, sel0_sb), (1, sel1_sb)):
                        nc.tensor.matmul(
                            rb_ps[:],
                            lhsT=sel[:],
                            rhs=sums_sb[0:1, 2 * p + j, qs],
                            start=(j == 0),
                            stop=(j == 1),
                        )
                    rb = ep.tile([128, QB], F32, tag="rb", name="rb")
                    nc.vector.reciprocal_approx_fast(out=rb[:], in_=rb_ps[:])
                    nc.vector.tensor_mul(OT[p][:, qs], OT[p][:, qs], rb[:])

            # ---------------- pipelined main loop ----------------
            late_ctx = ExitStack()
            latep = late_ctx.enter_context(tc.tile_pool(name="late", bufs=1))

            proj_block(0)
            pending_wo = []  # (qn, st) of deferred output projections
            for qn in range(NQ):
                nkt = 4 * qn + 4
                last_block = qn == NQ - 1
                av = [
                    avp.tile([DH + 1, QB], F32, tag="av", name=f"av{h}")
                    for h in range(HPC)
                ]
                prevE = None
                for kt in range(nkt):
                    if pending_wo and kt in (2, 5, 8, 11):
                        emit_wo(*pending_wo.pop(0), last_block=False)
                    E = emit_scores_exp(qn, kt)
                    if prevE is not None:
                        emit_av(qn, kt - 1, prevE, av, nkt)
                    prevE = E
                emit_av(qn, nkt - 1, prevE, av, nkt)
                # leftover deferred Wo (short blocks have few kt slots)
                while pending_wo:
                    emit_wo(*pending_wo.pop(0), last_block=False)
                if not last_block:
                    proj_block(qn + 1)
                epilogue_norm(qn, av)
                for st in range(4 * qn, 4 * qn + 4):
                    if last_block:
                        emit_wo(qn, st, last_block=True)
                    else:
                        pending_wo.append((qn, st))
            while pending_wo:
                emit_wo(*pending_wo.pop(0), last_block=True)
            late_ctx.close()
    return nc


def _get_nc():
    if "nc" not in _CACHE:
        nc = _build_nc()
        nc.finalize()
        _CACHE["nc"] = nc
    return _CACHE["nc"]


def _host_consts():
    rk = np.arange(KB)[:, None]
    rq = np.arange(KB)[None, :]
    tri = np.where(rq >= rk, 0.0, MASK_VAL).astype(ml_dtypes.bfloat16)
    identity = np.eye(KB, dtype=ml_dtypes.bfloat16)
    return tri, identity


def kernel(x, Wq, Wk, Wv, Wo):
    global LAST_RESULTS
    x = np.asarray(x, np.float32)
    Wq = np.asarray(Wq, np.float32)
    Wk = np.asarray(Wk, np.float32)
    Wv = np.asarray(Wv, np.float32)
    Wo = np.asarray(Wo, np.float32)

    pdt, wdt = _np_dt(_DT["proj"]), _np_dt(_DT["wo"])
    tri, identity = _host_consts()
    sel2_np = np.zeros((2, KB), np.float32)
    sel2_np[0, 0:64] = 1.0
    sel2_np[1, 64:128] = 1.0
    xTs = [np.ascontiguousarray(x[b].T).astype(pdt) for b in range(B)]

    in_maps = []
    for c in range(NCORES):
        b, g = c // (NCORES // B), c % (NCORES // B)
        jsel = slice(g * JPC, (g + 1) * JPC)
        in_maps.append(
            {
                "xT": xTs[b],
                "wqT": np.ascontiguousarray(Wq[jsel].T).astype(pdt),
                "wkT": np.ascontiguousarray(Wk[jsel].T).astype(pdt),
                "wvT": np.ascontiguousarray(Wv[jsel].T).astype(pdt),
                "woT": np.ascontiguousarray(Wo[:, jsel].T).astype(wdt),
                "tri": tri,
                "sel2": sel2_np,
                "ident": identity,
            }
        )

    res = run_bass_kernel_spmd(_get_nc(), in_maps, list(range(NCORES)))
    LAST_RESULTS = res
    ys = [res.results[c]["y"].astype(np.float32) for c in range(NCORES)]
    npc = NCORES // B
    out = np.stack(
        [sum(ys[b * npc + 1 : (b + 1) * npc], ys[b * npc]) for b in range(B)]
    )
    return out.astype(np.float32)


# revision 32
# speedup vs baseline: 1.5138x; 1.0218x over previous
"""Causal multi-head self-attention on 8 Trainium2 NeuronCores.

Problem (hardcoded): x [2, 2048, 1024] f32, Wq/Wk/Wv/Wo [1024, 1024] f32,
H=16 heads, Dh=64, causal softmax(QK^T/8)V then output projection.

Sharding (Megatron-style): 2-way data parallel over batch x 4-way tensor
parallel over heads.  Core c handles batch c//4 and heads 4*(c%4)..+3 (a
256-wide slice of the hidden dim).  Wq/Wk/Wv sliced column-wise, Wo
row-wise; each core emits a partial [2048, 1024] output (bf16) which the
host sums per batch.

Device dataflow per core (single fused pipeline, ascending q-blocks):
  - x^T supplied host-side (d on partitions); per-(dc, qn) column-sliced
    DMAs so block 0's inputs land first
  - per q-block qn: Q/K/V projections for the *next* block are batched
    between this block's attention and epilogue, so TensorE never waits
    on DMA and ScalarE exp overlaps projection matmuls
  - scores computed transposed S^T[k, q], 2 heads row-packed in the PE
    array (Dh=64 contraction); straddle (diagonal) key-tiles compute only
    the valid column range [lo:512)
  - triangular causal mask added into PSUM via identity-matmul of a
    single [128,128] bf16 tile, only on the 128-wide diagonal sub-block
  - one exp() per PSUM tile on ScalarE (scale=1/8 folded in; straddles
    use a single strided sliced activation)
  - A.V with stationary [V | ones] so the softmax denominator appears as
    row 64 of the same matmul output; straddle key-tiles write only the
    valid column range (start flag on kt=0 which is always full-width)
  - normalize: K=1 ones-matmul broadcast + DVE reciprocal + tensor_mul
  - Wo row-parallel per 128-row output tile; Wo matmuls of block qn are
    spread into block qn+1's key-tile loop to fill exp-wait bubbles
"""

import os
import sys
from contextlib import ExitStack

import numpy as np

try:
    import concourse.bass as bass
except ImportError:  # pragma: no cover - path fallback for fresh dirs
    for p in ("/opt/trn_rl_repo", "/root/.axon_site/_ro/trn_rl_repo"):
        if os.path.isdir(p) and p not in sys.path:
            sys.path.insert(0, p)
    import concourse.bass as bass

import ml_dtypes
import concourse.bacc as bacc
import concourse.mybir as mybir
import concourse.tile as tile
from concourse.bass_utils import run_bass_kernel_spmd

F32 = mybir.dt.float32
F32R = mybir.dt.float32r
BF16 = mybir.dt.bfloat16

KCFG = os.environ.get("KCFG", "faster")
_DT = {
    "fast": dict(proj=BF16, qk=BF16, av=BF16, wo=F32R),
    "faster": dict(proj=BF16, qk=BF16, av=BF16, wo=BF16),
}[KCFG]
KRECIP = os.environ.get("KRECIP", "fast")

B, S, D = 2, 2048, 1024
H, DH = 16, 64
NCORES = 8
HPC = 4          # heads per core
JPC = HPC * DH   # 256 hidden dims per core
QB = 512         # query block
KB = 128         # key tile
NQ = S // QB     # 4
NK = S // KB     # 16
MASK_VAL = -1e7

_CACHE = {}
LAST_RESULTS = None


def _np_dt(dt):
    return ml_dtypes.bfloat16 if dt == BF16 else np.float32


def _build_nc():
    proj_dt, qk_dt, av_dt, wo_dt = _DT["proj"], _DT["qk"], _DT["av"], _DT["wo"]
    nc = bacc.Bacc()
    xT = nc.dram_tensor("xT", [D, S], proj_dt, kind="ExternalInput")
    wqT = nc.dram_tensor("wqT", [D, JPC], proj_dt, kind="ExternalInput")
    wkT = nc.dram_tensor("wkT", [D, JPC], proj_dt, kind="ExternalInput")
    wvT = nc.dram_tensor("wvT", [D, JPC], proj_dt, kind="ExternalInput")
    woT = nc.dram_tensor("woT", [JPC, D], wo_dt, kind="ExternalInput")
    tri = nc.dram_tensor("tri", [KB, KB], BF16, kind="ExternalInput")
    sel2 = nc.dram_tensor("sel2", [2, KB], F32R, kind="ExternalInput")
    ident = nc.dram_tensor("ident", [KB, KB], BF16, kind="ExternalInput")
    y = nc.dram_tensor("y", [S, D], BF16, kind="ExternalOutput")

    with tile.TileContext(nc) as tc:
        with (
            tc.tile_pool(name="const", bufs=1) as constp,
            tc.tile_pool(name="act", bufs=1) as actp,
            tc.tile_pool(name="e", bufs=8) as ep,
            tc.tile_pool(name="ps", bufs=2, space="PSUM") as psp,
            tc.tile_pool(name="avp", bufs=4, space="PSUM") as avp,
        ):
            ident_sb = constp.tile([KB, KB], BF16)
            tri_sb = constp.tile([KB, KB], BF16)
            wo_sb = actp.tile([128, 2, D], wo_dt)
            xT_sb = actp.tile([128, 8, S], proj_dt)
            wq_sb = actp.tile([128, 8, JPC], proj_dt)
            wk_sb = actp.tile([128, 8, JPC], proj_dt)
            wv_sb = actp.tile([128, 8, JPC], proj_dt)
            # QT/KT: [128, S] pair tiles; rows 0:64 head 2*pi, 64:128 2*pi+1
            QT = [actp.tile([128, S], qk_dt, name=f"QT{i}") for i in range(2)]
            KT = [actp.tile([128, S], qk_dt, name=f"KT{i}") for i in range(2)]
            # V with ones column appended per (k-tile, head): the softmax
            # denominator falls out of the A.V matmul as row 64
            V1 = actp.tile([128, NK, HPC, DH + 1], av_dt)
            OT = [actp.tile([128, S], av_dt, name=f"OT{i}") for i in range(2)]
            sums_sb = actp.tile([1, HPC, S], F32R, name="sums_sb")
            sel0_sb = constp.tile([1, KB], F32R)
            sel1_sb = constp.tile([1, KB], F32R)

            # ---------------- DMA issue ----------------
            # Per-queue DMA bandwidth is ~110GB/s: spread the 6MB of input
            # across sync/scalar/gpsimd/vector so block 0's operands (wq,
            # wk, wv, xT qn0, tri) land in parallel.
            wqR = wqT.rearrange("(c p) j -> p c j", p=128)
            wkR = wkT.rearrange("(c p) j -> p c j", p=128)
            nc.sync.dma_start(out=wq_sb[:, :, 0:128], in_=wqR[:, :, 0:128])
            nc.scalar.dma_start(out=wq_sb[:, :, 128:256], in_=wqR[:, :, 128:256])
            nc.gpsimd.dma_start(
                out=wv_sb[:], in_=wvT.rearrange("(c p) j -> p c j", p=128)
            )
            nc.gpsimd.dma_start(out=tri_sb[:], in_=tri[:])
            nc.gpsimd.dma_start(out=ident_sb[:], in_=ident[:])
            nc.gpsimd.dma_start(out=sel0_sb[:], in_=sel2[0:1, :])
            nc.gpsimd.dma_start(out=sel1_sb[:], in_=sel2[1:2, :])
            nc.gpsimd.memset(V1[:, :, :, DH : DH + 1], 1.0)
            for dc in range(8):
                eng = nc.sync if dc % 2 == 0 else nc.scalar
                eng.dma_start(
                    out=xT_sb[:, dc, 0:QB],
                    in_=xT[dc * 128 : (dc + 1) * 128, 0:QB],
                )
            nc.sync.dma_start(out=wk_sb[:, :, 0:128], in_=wkR[:, :, 0:128])
            nc.scalar.dma_start(out=wk_sb[:, :, 128:256], in_=wkR[:, :, 128:256])
            for dc in range(8):
                eng = nc.sync if dc % 2 == 0 else nc.scalar
                eng.dma_start(
                    out=xT_sb[:, dc, QB : 2 * QB],
                    in_=xT[dc * 128 : (dc + 1) * 128, QB : 2 * QB],
                )
            nc.gpsimd.dma_start(
                out=wo_sb[:], in_=woT.rearrange("(c p) j -> p c j", p=128)
            )
            for dc in range(8):
                nc.gpsimd.dma_start(
                    out=xT_sb[:, dc, 2 * QB : 3 * QB],
                    in_=xT[dc * 128 : (dc + 1) * 128, 2 * QB : 3 * QB],
                )
            for dc in range(8):
                nc.sync.dma_start(
                    out=xT_sb[:, dc, 3 * QB : 4 * QB],
                    in_=xT[dc * 128 : (dc + 1) * 128, 3 * QB : 4 * QB],
                )

            # ---------------- helpers ----------------
            def qk_group(w_sb, out_tiles, mj, qn):
                ps = psp.tile([128, 1024], F32, tag="mm", name="ps_qk")
                for dc in range(8):
                    nc.tensor.matmul(
                        ps[:, :QB],
                        lhsT=w_sb[:, dc, mj * 128 : (mj + 1) * 128],
                        rhs=xT_sb[:, dc, qn * QB : (qn + 1) * QB],
                        start=(dc == 0),
                        stop=(dc == 7),
                    )
                nc.vector.tensor_copy(
                    out_tiles[mj][:, qn * QB : (qn + 1) * QB], ps[:, :QB]
                )

            def v_group(st):
                ps = psp.tile([128, 1024], F32, tag="mm", name="ps_v")
                for dc in range(8):
                    nc.tensor.matmul(
                        ps[:, :JPC],
                        lhsT=xT_sb[:, dc, st * 128 : (st + 1) * 128],
                        rhs=wv_sb[:, dc, :],
                        start=(dc == 0),
                        stop=(dc == 7),
                    )
                nc.vector.tensor_copy(
                    V1[:, st, :, 0:DH],
                    ps[:, :JPC].rearrange("p (h d) -> p h d", h=HPC),
                )

            def qk_block(qn):
                for mj in range(2):
                    qk_group(wq_sb, QT, mj, qn)
                for mj in range(2):
                    qk_group(wk_sb, KT, mj, qn)

            def emit_scores_exp(qn, kt):
                """Returns E tile pair for this key tile."""
                straddle = kt >= 4 * qn
                lo = 128 * (kt - 4 * qn) if straddle else 0
                E = []
                for pi in range(2):
                    ps = psp.tile([128, 1024], F32, tag="mm", name="ps_sc")
                    for hh in range(2):
                        nc.tensor.matmul(
                            ps[:, hh * QB + lo : (hh + 1) * QB],
                            lhsT=KT[pi][
                                hh * 64 : (hh + 1) * 64,
                                kt * KB : (kt + 1) * KB,
                            ],
                            rhs=QT[pi][
                                hh * 64 : (hh + 1) * 64,
                                qn * QB + lo : (qn + 1) * QB,
                            ],
                            start=True,
                            stop=not straddle,
                            tile_position=(hh * 64, 0),
                        )
                    if straddle:
                        for hh in range(2):
                            nc.tensor.matmul(
                                ps[:, hh * QB + lo : hh * QB + lo + 128],
                                lhsT=ident_sb,
                                rhs=tri_sb,
                                start=False,
                                stop=True,
                            )
                    e = ep.tile([128, 1024], av_dt, tag="e", name="e")
                    # one contiguous sliced exp [lo:1024] per tile: ScalarE
                    # ACTIVATE costs ~345ns + 0.74ns/col, so a single big op
                    # beats per-hh slices.  Columns in [512, 512+lo) are
                    # exp(stale PSUM) that the sliced A.V never reads.
                    nc.scalar.activation(
                        e[:, lo:],
                        ps[:, lo:],
                        mybir.ActivationFunctionType.Exp,
                        scale=0.125,
                    )
                    E.append(e)
                return E

            def emit_av(qn, kt, E, av, nkt):
                straddle = kt >= 4 * qn
                lo = 128 * (kt - 4 * qn) if straddle else 0
                first, last = kt == 0, kt == nkt - 1
                for h in range(HPC):
                    p, j = h // 2, h % 2
                    nc.tensor.matmul(
                        av[h][:, lo:QB],
                        lhsT=V1[:, kt, h, :],
                        rhs=E[p][:, j * QB + lo : (j + 1) * QB],
                        start=first,
                        stop=last,
                    )

            def emit_wo(qn, st, last_block):
                ps_y = psp.tile([128, 1024], F32, tag="mm", name="ps_y")
                for nn in range(2):
                    for p in range(2):
                        nc.tensor.matmul(
                            ps_y[:, nn * QB : (nn + 1) * QB],
                            lhsT=OT[p][:, st * 128 : (st + 1) * 128],
                            rhs=wo_sb[:, p, nn * QB : (nn + 1) * QB],
                            start=(p == 0),
                            stop=(p == 1),
                        )
                y_sb = latep.tile([128, D], av_dt, tag="y", bufs=3, name="y_sb")
                if last_block:
                    # ScalarE is free of exps by now; spread the final DMAs
                    # over all three queues so the drain isn't queue-bound
                    nc.scalar.copy(y_sb[:], ps_y[:])
                    oeng = (nc.sync, nc.scalar, nc.gpsimd)[st % 3]
                else:
                    nc.vector.tensor_copy(y_sb[:], ps_y[:])
                    oeng = nc.sync if st % 2 == 0 else nc.scalar
                oeng.dma_start(out=y[st * 128 : (st + 1) * 128, :], in_=y_sb[:])

            def epilogue_norm(qn, av, last_block):
                qs = slice(qn * QB, (qn + 1) * QB)
                # sums copies first so the broadcast matmuls start while the
                # (longer) OT copies still run on DVE
                for h in range(HPC):
                    nc.vector.tensor_copy(
                        sums_sb[0:1, h, qs], av[h][DH : DH + 1, :]
                    )
                # broadcast the raw sums to [0:64]/[64:128] with two K=1
                # selector matmuls, then one 128-lane approx reciprocal
                # (single-partition DVE reciprocals run on 1 lane: ~4us!)
                rb_pss = []
                for p in range(2):
                    rb_ps = avp.tile([128, QB], F32, tag="av", name="rb_ps")
                    for j, sel in ((0, sel0_sb), (1, sel1_sb)):
                        nc.tensor.matmul(
                            rb_ps[:],
                            lhsT=sel[:],
                            rhs=sums_sb[0:1, 2 * p + j, qs],
                            start=(j == 0),
                            stop=(j == 1),
                        )
                    rb_pss.append(rb_ps)
                for h in range(HPC):
                    p, j = h // 2, h % 2
                    nc.vector.tensor_copy(
                        OT[p][j * 64 : (j + 1) * 64, qs], av[h][0:DH, :]
                    )
                rbs = []
                for p in range(2):
                    rb = ep.tile([128, QB], F32, tag="rb", name="rb")
                    nc.vector.reciprocal_approx_fast(
                        out=rb[:], in_=rb_pss[p][:]
                    )
                    rbs.append(rb)
                if last_block:
                    # per-st normalize so each Wo + output DMA starts as
                    # early as possible in the drain
                    for st in range(4 * qn, 4 * qn + 4):
                        cs = slice(st * 128, (st + 1) * 128)
                        co = slice((st - 4 * qn) * 128, (st - 4 * qn + 1) * 128)
                        for p in range(2):
                            nc.vector.tensor_mul(
                                OT[p][:, cs], OT[p][:, cs], rbs[p][:, co]
                            )
                        emit_wo(qn, st, last_block=True)
                else:
                    for p in range(2):
                        nc.vector.tensor_mul(
                            OT[p][:, qs], OT[p][:, qs], rbs[p][:]
                        )

            # ---------------- pipelined main loop ----------------
            late_ctx = ExitStack()
            latep = late_ctx.enter_context(tc.tile_pool(name="late", bufs=1))

            qk_block(0)
            pending_wo = []  # (qn, st) of deferred output projections
            pending_v = list(range(4))  # st of deferred V projections
            for qn in range(NQ):
                nkt = 4 * qn + 4
                last_block = qn == NQ - 1
                av = [
                    avp.tile([DH + 1, QB], F32, tag="av", name=f"av{h}")
                    for h in range(HPC)
                ]
                prevE = None
                for kt in range(nkt):
                    # PE filler between exp-gated attention steps: this
                    # block's own V projections just-in-time, then the
                    # previous block's deferred Wo tiles
                    if pending_v and kt in (1, 2, 3, 4):
                        v_group(pending_v.pop(0))
                    if pending_wo and kt in (5, 7, 9, 11):
                        emit_wo(*pending_wo.pop(0), last_block=False)
                    E = emit_scores_exp(qn, kt)
                    if prevE is not None:
                        emit_av(qn, kt - 1, prevE, av, nkt)
                    prevE = E
                while pending_v:  # block 0 has fewer kt slots than groups
                    v_group(pending_v.pop(0))
                emit_av(qn, nkt - 1, prevE, av, nkt)
                while pending_wo:
                    emit_wo(*pending_wo.pop(0), last_block=False)
                if not last_block:
                    qk_block(qn + 1)
                    pending_v = list(range(4 * qn + 4, 4 * qn + 8))
                epilogue_norm(qn, av, last_block)
                if not last_block:
                    for st in range(4 * qn, 4 * qn + 4):
                        pending_wo.append((qn, st))
            late_ctx.close()
    return nc


def _get_nc():
    if "nc" not in _CACHE:
        nc = _build_nc()
        nc.finalize()
        _CACHE["nc"] = nc
    return _CACHE["nc"]


def _host_consts():
    rk = np.arange(KB)[:, None]
    rq = np.arange(KB)[None, :]
    tri = np.where(rq >= rk, 0.0, MASK_VAL).astype(ml_dtypes.bfloat16)
    identity = np.eye(KB, dtype=ml_dtypes.bfloat16)
    return tri, identity


def kernel(x, Wq, Wk, Wv, Wo):
    global LAST_RESULTS
    x = np.asarray(x, np.float32)
    Wq = np.asarray(Wq, np.float32)
    Wk = np.asarray(Wk, np.float32)
    Wv = np.asarray(Wv, np.float32)
    Wo = np.asarray(Wo, np.float32)

    pdt, wdt = _np_dt(_DT["proj"]), _np_dt(_DT["wo"])
    tri, identity = _host_consts()
    sel2_np = np.zeros((2, KB), np.float32)
    sel2_np[0, 0:64] = 1.0
    sel2_np[1, 64:128] = 1.0
    xTs = [np.ascontiguousarray(x[b].T).astype(pdt) for b in range(B)]

    in_maps = []
    for c in range(NCORES):
        b, g = c // (NCORES // B), c % (NCORES // B)
        jsel = slice(g * JPC, (g + 1) * JPC)
        in_maps.append(
            {
                "xT": xTs[b],
                "wqT": np.ascontiguousarray(Wq[jsel].T).astype(pdt),
                "wkT": np.ascontiguousarray(Wk[jsel].T).astype(pdt),
                "wvT": np.ascontiguousarray(Wv[jsel].T).astype(pdt),
                "woT": np.ascontiguousarray(Wo[:, jsel].T).astype(wdt),
                "tri": tri,
                "sel2": sel2_np,
                "ident": identity,
            }
        )

    res = run_bass_kernel_spmd(_get_nc(), in_maps, list(range(NCORES)))
    LAST_RESULTS = res
    ys = [res.results[c]["y"].astype(np.float32) for c in range(NCORES)]
    npc = NCORES // B
    out = np.stack(
        [sum(ys[b * npc + 1 : (b + 1) * npc], ys[b * npc]) for b in range(B)]
    )
    return out.astype(np.float32)


# revision 34
# speedup vs baseline: 1.5269x; 1.0086x over previous
"""Causal multi-head self-attention on 8 Trainium2 NeuronCores.

Problem (hardcoded): x [2, 2048, 1024] f32, Wq/Wk/Wv/Wo [1024, 1024] f32,
H=16 heads, Dh=64, causal softmax(QK^T/8)V then output projection.

Sharding (Megatron-style): 2-way data parallel over batch x 4-way tensor
parallel over heads.  Core c handles batch c//4 and heads 4*(c%4)..+3 (a
256-wide slice of the hidden dim).  Wq/Wk/Wv sliced column-wise, Wo
row-wise; each core emits a partial [2048, 1024] output (bf16) which the
host sums per batch.

Device dataflow per core (single fused pipeline, ascending q-blocks):
  - x^T supplied host-side (d on partitions); per-(dc, qn) column-sliced
    DMAs so block 0's inputs land first
  - per q-block qn: Q/K/V projections for the *next* block are batched
    between this block's attention and epilogue, so TensorE never waits
    on DMA and ScalarE exp overlaps projection matmuls
  - scores computed transposed S^T[k, q], 2 heads row-packed in the PE
    array (Dh=64 contraction); straddle (diagonal) key-tiles compute only
    the valid column range [lo:512)
  - triangular causal mask added into PSUM via identity-matmul of a
    single [128,128] bf16 tile, only on the 128-wide diagonal sub-block
  - one exp() per PSUM tile on ScalarE (scale=1/8 folded in; straddles
    use a single strided sliced activation)
  - A.V with stationary [V | ones] so the softmax denominator appears as
    row 64 of the same matmul output; straddle key-tiles write only the
    valid column range (start flag on kt=0 which is always full-width)
  - normalize: K=1 ones-matmul broadcast + DVE reciprocal + tensor_mul
  - Wo row-parallel per 128-row output tile; Wo matmuls of block qn are
    spread into block qn+1's key-tile loop to fill exp-wait bubbles
"""

import os
import sys
from contextlib import ExitStack

import numpy as np

try:
    import concourse.bass as bass
except ImportError:  # pragma: no cover - path fallback for fresh dirs
    for p in ("/opt/trn_rl_repo", "/root/.axon_site/_ro/trn_rl_repo"):
        if os.path.isdir(p) and p not in sys.path:
            sys.path.insert(0, p)
    import concourse.bass as bass

import ml_dtypes
import concourse.bacc as bacc
import concourse.mybir as mybir
import concourse.tile as tile
from concourse.bass_utils import run_bass_kernel_spmd

F32 = mybir.dt.float32
F32R = mybir.dt.float32r
BF16 = mybir.dt.bfloat16

KCFG = os.environ.get("KCFG", "faster")
_DT = {
    "fast": dict(proj=BF16, qk=BF16, av=BF16, wo=F32R),
    "faster": dict(proj=BF16, qk=BF16, av=BF16, wo=BF16),
}[KCFG]
KRECIP = os.environ.get("KRECIP", "fast")

B, S, D = 2, 2048, 1024
H, DH = 16, 64
NCORES = 8
HPC = 4          # heads per core
JPC = HPC * DH   # 256 hidden dims per core
QB = 512         # query block
KB = 128         # key tile
NQ = S // QB     # 4
NK = S // KB     # 16
MASK_VAL = -1e7

_CACHE = {}
LAST_RESULTS = None


def _np_dt(dt):
    return ml_dtypes.bfloat16 if dt == BF16 else np.float32


def _build_nc():
    proj_dt, qk_dt, av_dt, wo_dt = _DT["proj"], _DT["qk"], _DT["av"], _DT["wo"]
    nc = bacc.Bacc()
    xT = nc.dram_tensor("xT", [D, S], proj_dt, kind="ExternalInput")
    wqT = nc.dram_tensor("wqT", [D, JPC], proj_dt, kind="ExternalInput")
    wkT = nc.dram_tensor("wkT", [D, JPC], proj_dt, kind="ExternalInput")
    wvT = nc.dram_tensor("wvT", [D, JPC], proj_dt, kind="ExternalInput")
    woT = nc.dram_tensor("woT", [JPC, D], wo_dt, kind="ExternalInput")
    tri2 = nc.dram_tensor("tri2", [KB, 2, KB], BF16, kind="ExternalInput")
    sel2 = nc.dram_tensor("sel2", [2, KB], F32R, kind="ExternalInput")
    y = nc.dram_tensor("y", [S, D], BF16, kind="ExternalOutput")

    with tile.TileContext(nc) as tc:
        with (
            tc.tile_pool(name="const", bufs=1) as constp,
            tc.tile_pool(name="act", bufs=1) as actp,
            tc.tile_pool(name="e", bufs=8) as ep,
            tc.tile_pool(name="ps", bufs=2, space="PSUM") as psp,
            tc.tile_pool(name="avp", bufs=4, space="PSUM") as avp,
        ):
            tri2_sb = constp.tile([KB, 2, KB], BF16)
            wo_sb = actp.tile([128, 2, D], wo_dt)
            xT_sb = actp.tile([128, 8, S], proj_dt)
            wq_sb = actp.tile([128, 8, JPC], proj_dt)
            wk_sb = actp.tile([128, 8, JPC], proj_dt)
            wv_sb = actp.tile([128, 8, JPC], proj_dt)
            # QT/KT: [128, S] pair tiles; rows 0:64 head 2*pi, 64:128 2*pi+1
            QT = [actp.tile([128, S], qk_dt, name=f"QT{i}") for i in range(2)]
            KT = [actp.tile([128, S], qk_dt, name=f"KT{i}") for i in range(2)]
            # V with ones column appended per (k-tile, head): the softmax
            # denominator falls out of the A.V matmul as row 64
            V1 = actp.tile([128, NK, HPC, DH + 1], av_dt)
            OT = [actp.tile([128, S], av_dt, name=f"OT{i}") for i in range(2)]
            sums_sb = actp.tile([1, HPC, S], F32R, name="sums_sb")
            sel0_sb = constp.tile([1, KB], F32R)
            sel1_sb = constp.tile([1, KB], F32R)

            # ---------------- DMA issue ----------------
            # Per-queue DMA bandwidth is ~110GB/s: spread the 6MB of input
            # across sync/scalar/gpsimd/vector so block 0's operands (wq,
            # wk, wv, xT qn0, tri) land in parallel.
            wqR = wqT.rearrange("(c p) j -> p c j", p=128)
            wkR = wkT.rearrange("(c p) j -> p c j", p=128)
            nc.sync.dma_start(out=wq_sb[:, :, 0:128], in_=wqR[:, :, 0:128])
            nc.scalar.dma_start(out=wq_sb[:, :, 128:256], in_=wqR[:, :, 128:256])
            nc.gpsimd.dma_start(
                out=wv_sb[:], in_=wvT.rearrange("(c p) j -> p c j", p=128)
            )
            nc.gpsimd.dma_start(out=tri2_sb[:], in_=tri2[:])
            nc.gpsimd.dma_start(out=sel0_sb[:], in_=sel2[0:1, :])
            nc.gpsimd.dma_start(out=sel1_sb[:], in_=sel2[1:2, :])
            nc.gpsimd.memset(V1[:, :, :, DH : DH + 1], 1.0)
            for dc in range(8):
                eng = nc.sync if dc % 2 == 0 else nc.scalar
                eng.dma_start(
                    out=xT_sb[:, dc, 0:QB],
                    in_=xT[dc * 128 : (dc + 1) * 128, 0:QB],
                )
            nc.sync.dma_start(out=wk_sb[:, :, 0:128], in_=wkR[:, :, 0:128])
            nc.scalar.dma_start(out=wk_sb[:, :, 128:256], in_=wkR[:, :, 128:256])
            for dc in range(8):
                eng = nc.sync if dc % 2 == 0 else nc.scalar
                eng.dma_start(
                    out=xT_sb[:, dc, QB : 2 * QB],
                    in_=xT[dc * 128 : (dc + 1) * 128, QB : 2 * QB],
                )
            nc.gpsimd.dma_start(
                out=wo_sb[:], in_=woT.rearrange("(c p) j -> p c j", p=128)
            )
            for dc in range(8):
                nc.gpsimd.dma_start(
                    out=xT_sb[:, dc, 2 * QB : 3 * QB],
                    in_=xT[dc * 128 : (dc + 1) * 128, 2 * QB : 3 * QB],
                )
            for dc in range(8):
                nc.sync.dma_start(
                    out=xT_sb[:, dc, 3 * QB : 4 * QB],
                    in_=xT[dc * 128 : (dc + 1) * 128, 3 * QB : 4 * QB],
                )

            # ---------------- helpers ----------------
            def qk_group(w_sb, out_tiles, mj, qn):
                ps = psp.tile([128, 1024], F32, tag="mm", name="ps_qk")
                for dc in range(8):
                    nc.tensor.matmul(
                        ps[:, :QB],
                        lhsT=w_sb[:, dc, mj * 128 : (mj + 1) * 128],
                        rhs=xT_sb[:, dc, qn * QB : (qn + 1) * QB],
                        start=(dc == 0),
                        stop=(dc == 7),
                    )
                nc.vector.tensor_copy(
                    out_tiles[mj][:, qn * QB : (qn + 1) * QB], ps[:, :QB]
                )

            def v_group(st):
                ps = psp.tile([128, 1024], F32, tag="mm", name="ps_v")
                for dc in range(8):
                    nc.tensor.matmul(
                        ps[:, :JPC],
                        lhsT=xT_sb[:, dc, st * 128 : (st + 1) * 128],
                        rhs=wv_sb[:, dc, :],
                        start=(dc == 0),
                        stop=(dc == 7),
                    )
                nc.vector.tensor_copy(
                    V1[:, st, :, 0:DH],
                    ps[:, :JPC].rearrange("p (h d) -> p h d", h=HPC),
                )

            def qk_block(qn):
                for mj in range(2):
                    qk_group(wq_sb, QT, mj, qn)
                for mj in range(2):
                    qk_group(wk_sb, KT, mj, qn)

            def emit_scores_exp(qn, kt):
                """Returns E tile pair for this key tile."""
                straddle = kt >= 4 * qn
                lo = 128 * (kt - 4 * qn) if straddle else 0
                E = []
                for pi in range(2):
                    ps = psp.tile([128, 1024], F32, tag="mm", name="ps_sc")
                    for hh in range(2):
                        nc.tensor.matmul(
                            ps[:, hh * QB + lo : (hh + 1) * QB],
                            lhsT=KT[pi][
                                hh * 64 : (hh + 1) * 64,
                                kt * KB : (kt + 1) * KB,
                            ],
                            rhs=QT[pi][
                                hh * 64 : (hh + 1) * 64,
                                qn * QB + lo : (qn + 1) * QB,
                            ],
                            start=True,
                            stop=True,
                            tile_position=(hh * 64, 0),
                        )
                    e = ep.tile([128, 1024], av_dt, tag="e", name="e")
                    # one contiguous sliced exp [lo:1024] per tile: ScalarE
                    # ACTIVATE costs ~345ns + 0.74ns/col, so a single big op
                    # beats per-hh slices.  Columns in [512, 512+lo) are
                    # exp(stale PSUM) that the sliced A.V never reads.
                    nc.scalar.activation(
                        e[:, lo:],
                        ps[:, lo:],
                        mybir.ActivationFunctionType.Exp,
                        scale=0.125,
                    )
                    if straddle:
                        # zero the upper triangle of the diagonal 128-col
                        # sub-block on DVE (cheaper than PE identity-matmul
                        # mask adds: those paid an ident LDW per matmul)
                        ev = e[:].rearrange("p (h q) -> p h q", h=2)
                        nc.vector.tensor_mul(
                            ev[:, :, lo : lo + 128],
                            ev[:, :, lo : lo + 128],
                            tri2_sb[:],
                        )
                    E.append(e)
                return E

            def emit_av(qn, kt, E, av, nkt):
                straddle = kt >= 4 * qn
                lo = 128 * (kt - 4 * qn) if straddle else 0
                first, last = kt == 0, kt == nkt - 1
                for h in range(HPC):
                    p, j = h // 2, h % 2
                    nc.tensor.matmul(
                        av[h][:, lo:QB],
                        lhsT=V1[:, kt, h, :],
                        rhs=E[p][:, j * QB + lo : (j + 1) * QB],
                        start=first,
                        stop=last,
                    )

            def emit_wo(qn, st, last_block):
                ps_y = psp.tile([128, 1024], F32, tag="mm", name="ps_y")
                for nn in range(2):
                    for p in range(2):
                        nc.tensor.matmul(
                            ps_y[:, nn * QB : (nn + 1) * QB],
                            lhsT=OT[p][:, st * 128 : (st + 1) * 128],
                            rhs=wo_sb[:, p, nn * QB : (nn + 1) * QB],
                            start=(p == 0),
                            stop=(p == 1),
                        )
                y_sb = latep.tile([128, D], av_dt, tag="y", bufs=3, name="y_sb")
                if last_block:
                    # ScalarE is free of exps by now; split each tile into
                    # halves over all three queues so the drain overlaps
                    nc.scalar.copy(y_sb[:], ps_y[:])
                    for u in range(2):
                        oeng = (nc.sync, nc.scalar, nc.gpsimd)[(2 * st + u) % 3]
                        oeng.dma_start(
                            out=y[st * 128 : (st + 1) * 128, u * QB : (u + 1) * QB],
                            in_=y_sb[:, u * QB : (u + 1) * QB],
                        )
                else:
                    nc.vector.tensor_copy(y_sb[:], ps_y[:])
                    oeng = nc.sync if st % 2 == 0 else nc.scalar
                    oeng.dma_start(
                        out=y[st * 128 : (st + 1) * 128, :], in_=y_sb[:]
                    )

            def epilogue_norm(qn, av, last_block):
                qs = slice(qn * QB, (qn + 1) * QB)
                # sums copies first so the broadcast matmuls start while the
                # (longer) OT copies still run on DVE
                for h in range(HPC):
                    nc.vector.tensor_copy(
                        sums_sb[0:1, h, qs], av[h][DH : DH + 1, :]
                    )
                # broadcast the raw sums to [0:64]/[64:128] with two K=1
                # selector matmuls, then one 128-lane approx reciprocal
                # (single-partition DVE reciprocals run on 1 lane: ~4us!)
                rb_tile = psp.tile([128, 1024], F32, tag="mm", name="rb_ps")
                for p in range(2):
                    for j, sel in ((0, sel0_sb), (1, sel1_sb)):
                        nc.tensor.matmul(
                            rb_tile[:, p * QB : (p + 1) * QB],
                            lhsT=sel[:],
                            rhs=sums_sb[0:1, 2 * p + j, qs],
                            start=(j == 0),
                            stop=(j == 1),
                        )
                rbs = []
                for p in range(2):
                    rb = ep.tile([128, QB], F32, tag="rb", name="rb")
                    nc.vector.reciprocal_approx_fast(
                        out=rb[:], in_=rb_tile[:, p * QB : (p + 1) * QB]
                    )
                    rbs.append(rb)
                # fused normalize-copy: OT = av * (1/sums) straight out of
                # PSUM, one tensor_tensor per head
                if last_block:
                    # per-st so each Wo + output DMA starts as early as
                    # possible in the drain
                    for st in range(4 * qn, 4 * qn + 4):
                        cs = slice(st * 128, (st + 1) * 128)
                        co = slice((st - 4 * qn) * 128, (st - 4 * qn + 1) * 128)
                        for h in range(HPC):
                            p, j = h // 2, h % 2
                            nc.vector.tensor_mul(
                                OT[p][j * 64 : (j + 1) * 64, cs],
                                av[h][0:DH, co],
                                rbs[p][j * 64 : (j + 1) * 64, co],
                            )
                        emit_wo(qn, st, last_block=True)
                else:
                    for h in range(HPC):
                        p, j = h // 2, h % 2
                        nc.vector.tensor_mul(
                            OT[p][j * 64 : (j + 1) * 64, qs],
                            av[h][0:DH, :],
                            rbs[p][j * 64 : (j + 1) * 64, :],
                        )

            # ---------------- pipelined main loop ----------------
            late_ctx = ExitStack()
            latep = late_ctx.enter_context(tc.tile_pool(name="late", bufs=1))

            qk_block(0)
            pending_wo = []  # (qn, st) of deferred output projections
            pending_v = list(range(4))  # st of deferred V projections
            for qn in range(NQ):
                nkt = 4 * qn + 4
                last_block = qn == NQ - 1
                av = [
                    avp.tile([DH + 1, QB], F32, tag="av", name=f"av{h}")
                    for h in range(HPC)
                ]
                prevE = None
                for kt in range(nkt):
                    # PE filler between exp-gated attention steps: this
                    # block's own V projections just-in-time, then the
                    # previous block's deferred Wo tiles
                    if pending_v and kt in (1, 2, 3, 4):
                        v_group(pending_v.pop(0))
                    if pending_wo and kt in (5, 7, 9, 11):
                        emit_wo(*pending_wo.pop(0), last_block=False)
                    E = emit_scores_exp(qn, kt)
                    if prevE is not None:
                        emit_av(qn, kt - 1, prevE, av, nkt)
                    prevE = E
                while pending_v:  # block 0 has fewer kt slots than groups
                    v_group(pending_v.pop(0))
                emit_av(qn, nkt - 1, prevE, av, nkt)
                while pending_wo:
                    emit_wo(*pending_wo.pop(0), last_block=False)
                if not last_block:
                    qk_block(qn + 1)
                    pending_v = list(range(4 * qn + 4, 4 * qn + 8))
                epilogue_norm(qn, av, last_block)
                if not last_block:
                    for st in range(4 * qn, 4 * qn + 4):
                        pending_wo.append((qn, st))
            late_ctx.close()
    return nc


def _get_nc():
    if "nc" not in _CACHE:
        nc = _build_nc()
        nc.finalize()
        _CACHE["nc"] = nc
    return _CACHE["nc"]


def _host_consts():
    rk = np.arange(KB)[:, None]
    rq = np.arange(KB)[None, :]
    tri01 = np.where(rq >= rk, 1.0, 0.0).astype(ml_dtypes.bfloat16)
    tri2 = np.stack([tri01, tri01], axis=1)
    return np.ascontiguousarray(tri2)


def kernel(x, Wq, Wk, Wv, Wo):
    global LAST_RESULTS
    x = np.asarray(x, np.float32)
    Wq = np.asarray(Wq, np.float32)
    Wk = np.asarray(Wk, np.float32)
    Wv = np.asarray(Wv, np.float32)
    Wo = np.asarray(Wo, np.float32)

    pdt, wdt = _np_dt(_DT["proj"]), _np_dt(_DT["wo"])
    tri2_np = _host_consts()
    sel2_np = np.zeros((2, KB), np.float32)
    sel2_np[0, 0:64] = 1.0
    sel2_np[1, 64:128] = 1.0
    xTs = [np.ascontiguousarray(x[b].T).astype(pdt) for b in range(B)]

    in_maps = []
    for c in range(NCORES):
        b, g = c // (NCORES // B), c % (NCORES // B)
        jsel = slice(g * JPC, (g + 1) * JPC)
        in_maps.append(
            {
                "xT": xTs[b],
                "wqT": np.ascontiguousarray(Wq[jsel].T).astype(pdt),
                "wkT": np.ascontiguousarray(Wk[jsel].T).astype(pdt),
                "wvT": np.ascontiguousarray(Wv[jsel].T).astype(pdt),
                "woT": np.ascontiguousarray(Wo[:, jsel].T).astype(wdt),
                "tri2": tri2_np,
                "sel2": sel2_np,
            }
        )

    res = run_bass_kernel_spmd(_get_nc(), in_maps, list(range(NCORES)))
    LAST_RESULTS = res
    ys = [res.results[c]["y"].astype(np.float32) for c in range(NCORES)]
    npc = NCORES // B
    out = np.stack(
        [sum(ys[b * npc + 1 : (b + 1) * npc], ys[b * npc]) for b in range(B)]
    )
    return out.astype(np.float32)
